# revision 12
# baseline (speedup 1.0000x reference)
"""Trainium2 Bass kernel for nn_AttentionMap (B=4, H=16, S=2048, d=64, rel_d=32).

out[b,h,q,k] = softmax_k( clip(Q)·clip(K)^T * d^-.5 + clip(PQ)·clip(PK)^T * rd^-.5 )

Strategy (mode "v3"):
  - Shard the 64 (b,h) slices across 8 NeuronCores, 8 per core (data parallel,
    no collectives; softmax is over the local k axis).
  - Host-side prep (layout only): concat [q | sqrt(2)*pos_q] and [k | pos_k]
    along the feature dim (64+32=96), transpose each (b,h) slice to [96, S],
    cast to bf16.  Since 64^-.5 / 32^-.5 = 1/sqrt(2), scaling pos_q by sqrt(2)
    up front lets the single scale 1/8 ride the Exp activation's free affine —
    no per-element scaling pass on device.  The q columns are permuted so that
    PSUM partition p of score tile c holds query row 16p+c: partition p then
    accumulates output rows [16p, 16p+15] back-to-back in SBUF and the output
    DMA uses large contiguous per-partition descriptors.
  - Per (b,h) on device: clamp operands in place on VectorE (clip-then-scale
    == scale-then-clip-at-scaled-bounds), 4 bf16 matmuls per 128-row q tile
    into a [128,2048] PSUM tile, one ScalarE Exp (scale=1/8, NO accumulator —
    the accumulator readback costs 283ns/tile of ScalarE, the bottleneck
    engine) writing bf16, then VectorE: copy-to-f16 with accum_out giving the
    row sums (4x perf mode), reciprocal, and an in-place per-row scale of the
    f16 staging buffer.  Grouped 4-tile DMA to HBM.
  - Output is f16 (rel tolerance 2e-2 makes 16-bit storage safe); host
    converts back to f32.
"""
import numpy as np
import ml_dtypes
from contextlib import ExitStack

import concourse.tile as tile
from concourse import bacc, mybir
from concourse._compat import with_exitstack
from concourse.bass_utils import run_bass_kernel_spmd

F32 = mybir.dt.float32
F32R = mybir.dt.float32r
F16 = mybir.dt.float16
BF16 = mybir.dt.bfloat16

N_CORES = 8
B, H, S = 4, 16, 2048
DQ, DP = 64, 32
D = DQ + DP
SCALE = DQ ** -0.5          # 1/8
REL_SCALE = DP ** -0.5      # 1/(4*sqrt(2)) = sqrt(2)/8
RT2 = float(np.sqrt(2.0))
CLAMP = 5.0

MODE = "v7"
GROUP = 2                   # output tiles per DMA (8KB/partition descriptors)


@with_exitstack
def _attn_v4(ctx: ExitStack, tc: tile.TileContext, out_d, qt_d, kt_d,
             n_bh: int, s: int):
    nc = tc.nc
    n_ct = s // 128          # q tiles per bh
    n_kb = s // 512          # 512-wide k blocks per psum tile

    opnd = ctx.enter_context(tc.tile_pool(name="opnd", bufs=2))
    outp = ctx.enter_context(tc.tile_pool(name="outp", bufs=4))
    small = ctx.enter_context(tc.tile_pool(name="small", bufs=8))
    scps = ctx.enter_context(tc.tile_pool(name="scps", bufs=2, space="PSUM"))

    def prep(bh):
        """Load one bh's bf16 operands and clamp in place."""
        qs = opnd.tile([D, s], BF16, tag="qT")
        ks = opnd.tile([D, s], BF16, tag="kT")
        nc.sync.dma_start(out=qs[:], in_=qt_d[bh])
        nc.sync.dma_start(out=ks[:], in_=kt_d[bh])
        # content rows: clip at +-5 ; pos rows of q were pre-scaled by sqrt(2)
        # on host, so their clip bounds scale too.
        nc.vector.tensor_scalar(out=qs[:DQ, :], in0=qs[:DQ, :],
                                scalar1=CLAMP, scalar2=-CLAMP,
                                op0=mybir.AluOpType.min, op1=mybir.AluOpType.max)
        nc.vector.tensor_scalar(out=qs[DQ:, :], in0=qs[DQ:, :],
                                scalar1=CLAMP * RT2, scalar2=-CLAMP * RT2,
                                op0=mybir.AluOpType.min, op1=mybir.AluOpType.max)
        nc.vector.tensor_scalar(out=ks[:], in0=ks[:],
                                scalar1=CLAMP, scalar2=-CLAMP,
                                op0=mybir.AluOpType.min, op1=mybir.AluOpType.max)
        return qs, ks

    next_ops = prep(0)
    for bh in range(n_bh):
        (qs, ks), next_ops = next_ops, None
        obuf = None
        for c in range(n_ct):
            if c == n_ct // 2 and bh + 1 < n_bh:
                next_ops = prep(bh + 1)
            if c % GROUP == 0:
                obuf = outp.tile([128, GROUP * s], F16, tag="ob")
            sc = scps.tile([128, s], F32, tag="sc")
            for j in range(n_kb):
                cols = slice(j * 512, (j + 1) * 512)
                nc.tensor.matmul(sc[:, cols],
                                 lhsT=qs[:, c * 128:(c + 1) * 128],
                                 rhs=ks[:, cols], start=True, stop=True)
            ob = obuf[:, (c % GROUP) * s:(c % GROUP + 1) * s]
            tot = small.tile([128, 1], F32, tag="tot")
            # Exp straight into the f16 staging buffer.  Row sums are split
            # between the two engines that can produce them: ScalarE's
            # accumulator costs 283ns of the bottleneck engine per readback,
            # a VectorE reduce costs ~2.3us of the (slacker) vector engine —
            # 8/16 on ScalarE and 8/16 on VectorE balances both at ~268us.
            use_acc = (c % 2 == 0)
            nc.scalar.activation(out=ob, in_=sc[:],
                                 func=mybir.ActivationFunctionType.Exp,
                                 scale=SCALE,
                                 accum_out=tot[:] if use_acc else None)
            if not use_acc:
                nc.vector.tensor_reduce(out=tot[:], in_=ob,
                                        axis=mybir.AxisListType.X,
                                        op=mybir.AluOpType.add)
            rec = small.tile([128, 1], F32, tag="rec")
            nc.vector.reciprocal(out=rec[:], in_=tot[:])
            nc.vector.tensor_scalar_mul(out=ob, in0=ob, scalar1=rec[:])
            if c % GROUP == GROUP - 1:
                g = c // GROUP
                nc.sync.dma_start(
                    out=out_d[bh, :, g * GROUP * s:(g + 1) * GROUP * s],
                    in_=obuf[:])


def build(mode: str = MODE, n_bh: int = N_CORES, s: int = S):
    nc = bacc.Bacc("TRN2", target_bir_lowering=False, debug=False,
                   num_devices=N_CORES)
    qt_d = nc.dram_tensor("qt", [n_bh, D, s], BF16, kind="ExternalInput").ap()
    kt_d = nc.dram_tensor("kt", [n_bh, D, s], BF16, kind="ExternalInput").ap()
    # out[bh, p, c*s + k] holds softmax row q=16p+c: partition-major layout so
    # each partition's bytes are 16 consecutive q rows (64KB contiguous dram).
    out_d = nc.dram_tensor("out", [n_bh, 128, (s // 128) * s], F16,
                           kind="ExternalOutput").ap()
    with tile.TileContext(nc) as tc:
        _attn_v4(tc, out_d, qt_d, kt_d, n_bh, s)
    nc.compile()
    return nc


def _host_prep(keys, queries, pos_key, pos_query):
    """[B,H,S,d] inputs -> per-core {'qt','kt'} slices in [bh, 96, S] bf16.

    qt columns are permuted: column c*128+p holds query row 16p+c (so that
    score-tile c / PSUM partition p computes query row 16p+c).  pos_query is
    pre-scaled by sqrt(2) so a single 1/8 scale (folded into the device-side
    Exp) reproduces both SCALE and REL_SCALE.
    """
    qcat = np.concatenate([np.asarray(queries),
                           np.asarray(pos_query) * RT2], axis=-1)
    kcat = np.concatenate([np.asarray(keys), np.asarray(pos_key)], axis=-1)
    j = np.arange(S)
    perm = 16 * (j % 128) + j // 128
    qt = np.ascontiguousarray(
        qcat.reshape(B * H, S, D)[:, perm, :].swapaxes(1, 2)).astype(
            ml_dtypes.bfloat16)
    kt = np.ascontiguousarray(
        kcat.reshape(B * H, S, D).swapaxes(1, 2)).astype(ml_dtypes.bfloat16)
    per = (B * H) // N_CORES
    return [{"qt": qt[c * per:(c + 1) * per], "kt": kt[c * per:(c + 1) * per]}
            for c in range(N_CORES)]


def _run(keys, queries, pos_key, pos_query, mode=MODE, trace=False, **kw):
    in_maps = _host_prep(keys, queries, pos_key, pos_query)
    nc = build(mode=mode)
    res = run_bass_kernel_spmd(nc, in_maps, list(range(N_CORES)), trace=trace,
                               **kw)
    # out[bh, p, c*S+k] with q = 16p+c: a plain reshape restores q-major order.
    out = np.concatenate([res.results[c]["out"].reshape(-1, S, S)
                          for c in range(N_CORES)], axis=0)
    return out.reshape(B, H, S, S).astype(np.float32), res


def kernel(keys, queries, pos_key, pos_query):
    out, _ = _run(keys, queries, pos_key, pos_query)
    return out
